# revision 15
# baseline (speedup 1.0000x reference)
"""Trainium2 Bass kernel for nn_GCBlock (gnn_message_passing).

Data-parallel over batch (2048 -> 8 cores x 256). Per core, samples are
processed in 64 groups of 4, batched along the free dim in a transposed
(time-on-partition) layout. All matmuls bf16 (1 PE cycle/row), fp32 PSUM.
PSUM tiles are one bank each, double-buffered across groups.

Key structure per group:
- host ships x natural (PE stationary), x^T, and both banded shifted
  copies of x^T (roll +-1 over t), packed so each group needs only two
  big DMA loads (one 128-partition, one 66-partition) plus two stores.
- joint mixing (A1 + g2*A3) folded on host into per-sample AL (packed
  with natural x), applied as PE matmuls -> (AL@x)^T lands in PSUM.
- g1*x2 (banded adj_t) folded into a second FC matrix W2 = fc_w @ M2band;
  its stream input is g1-gated x^T (one vector op).
- g3*x4 (per-node banded adj_tj) from the pre-shifted x^T copies plus 4
  elementwise ops; summed into the (AL@x)^T stream during PSUM evac.
- FC = PSUM-accumulated matmuls over 2 streams x 2 k-halves.
- LN stats: mean via DVE segmented reduce, E[h^2] via Act square+accum;
  normalize F=0 on Act (per-sample scale/bias), F=1 on DVE broadcast APs.
- residual added in transposed layout; output shipped transposed and
  reassembled (transpose + f32 cast) on host.
"""
import numpy as np

B, V, T, J = 2048, 66, 256, 22
N_CORES = 8
BL = B // N_CORES          # 256 samples per core
NB = 4                     # samples per group
NG = BL // NB              # 64 groups
FD = NB * V                # 264 batched free dim

_NC_CACHE = {}


def _build_nc(trivial_affine):
    key = ("nc", trivial_affine)
    if key in _NC_CACHE:
        return _NC_CACHE[key]
    import concourse.bacc as bacc
    import concourse.mybir as mybir
    import concourse.tile as tile

    f32 = mybir.dt.float32
    bf16 = mybir.dt.bfloat16
    Alu = mybir.AluOpType
    Act = mybir.ActivationFunctionType

    nc = bacc.Bacc("TRN2", target_bir_lowering=False, debug=False,
                   num_devices=N_CORES)

    # packed big inputs: xbig[g] = [xT(h0), xT(h1), xlo(h0), xlo(h1),
    #                               xhi(h0), xhi(h1)] each [128, FD]
    xbig = nc.dram_tensor("xbig", [NG, 6, 128, FD], bf16,
                          kind="ExternalInput").ap()
    # xna[g] = [x natural (NB*T) | AL^T packed (FD)]
    xna = nc.dram_tensor("xna", [NG, V, NB * T + FD], bf16,
                         kind="ExternalInput").ap()
    gall = nc.dram_tensor("gall", [1, NG * 2 * FD], bf16,
                          kind="ExternalInput").ap()
    lohi = nc.dram_tensor("lohi", [2, 2, 128, FD], bf16,
                          kind="ExternalInput").ap()
    wqs = nc.dram_tensor("wqs", [2, 2, 2, 128, 128], bf16,
                         kind="ExternalInput").ap()
    arep = nc.dram_tensor("arep", [128, FD], bf16, kind="ExternalInput").ap()
    brep = nc.dram_tensor("brep", [128, FD], bf16, kind="ExternalInput").ap()
    fcb = nc.dram_tensor("fcb", [2, 128, 1], f32, kind="ExternalInput").ap()
    yst = nc.dram_tensor("yst", [NG, 2, 128, FD], bf16,
                         kind="ExternalOutput").ap()

    with tile.TileContext(nc) as tc:
        import contextlib
        with contextlib.ExitStack() as ctx:
            cpool = ctx.enter_context(tc.tile_pool(name="consts", bufs=1))
            xpool = ctx.enter_context(tc.tile_pool(name="xin", bufs=4))
            spool = ctx.enter_context(tc.tile_pool(name="sbwork", bufs=3))
            stpool = ctx.enter_context(tc.tile_pool(name="stats", bufs=3))
            pmx = ctx.enter_context(tc.tile_pool(name="psA", bufs=2, space="PSUM"))
            pph = ctx.enter_context(tc.tile_pool(name="psH", bufs=2, space="PSUM"))

            # ---- constants ----
            c_lo = [cpool.tile([128, FD], bf16, name=f"clo{h}", tag=f"clo{h}")
                    for h in range(2)]
            c_hi = [cpool.tile([128, FD], bf16, name=f"chi{h}", tag=f"chi{h}")
                    for h in range(2)]
            for h in range(2):
                nc.sync.dma_start(c_lo[h][:], lohi[0, h])
                nc.sync.dma_start(c_hi[h][:], lohi[1, h])
            c_wq = [[[cpool.tile([128, 128], bf16, name=f"cwq{w}{kh}{F}",
                                 tag=f"cwq{w}{kh}{F}")
                      for F in range(2)] for kh in range(2)] for w in range(2)]
            for w in range(2):
                for kh in range(2):
                    for F in range(2):
                        nc.sync.dma_start(c_wq[w][kh][F][:], wqs[w, kh, F])
            c_arep = cpool.tile([128, FD], bf16, name="carep", tag="carep")
            nc.sync.dma_start(c_arep[:], arep[:])
            c_brep = cpool.tile([128, FD], bf16, name="cbrep", tag="cbrep")
            nc.sync.dma_start(c_brep[:], brep[:])
            c_fcb = [cpool.tile([128, 1], f32, name=f"cfcb{F}", tag=f"cfcb{F}")
                     for F in range(2)]
            for F in range(2):
                nc.sync.dma_start(c_fcb[F][:], fcb[F])
            c_eps = cpool.tile([128, 1], f32, name="teps", tag="teps")
            nc.gpsimd.memset(c_eps[:], 1e-5)
            c_gall = cpool.tile([1, NG * 2 * FD], bf16, name="cgall",
                                tag="cgall")
            nc.sync.dma_start(c_gall[:], gall[:])

            for g in range(NG):
                # ---- loads (two big DMAs) ----
                xb = xpool.tile([128, 6 * FD], bf16, name="t10", tag="xb")
                nc.sync.dma_start(
                    xb[:].rearrange("p (s d) -> p s d", s=6),
                    xbig[g].rearrange("s p d -> p s d"))
                xv = xpool.tile([V, NB * T + FD], bf16, name="t11", tag="xv")
                nc.scalar.dma_start(xv[:], xna[g])
                sXT = [xb[:, FD * h:FD * (h + 1)] for h in range(2)]
                xlo = [xb[:, FD * (2 + h):FD * (3 + h)] for h in range(2)]
                xhi = [xb[:, FD * (4 + h):FD * (5 + h)] for h in range(2)]
                xn = xv[:, 0:NB * T]
                ab = xv[:, NB * T:NB * T + FD]

                g1rb = spool.tile([128, FD], bf16, name="t14", tag="g1rb")
                g3rb = spool.tile([128, FD], bf16, name="t15", tag="g3rb")
                goff = g * 2 * FD
                nc.gpsimd.partition_broadcast(
                    g1rb[:], c_gall[:, goff:goff + FD])
                nc.gpsimd.partition_broadcast(
                    g3rb[:], c_gall[:, goff + FD:goff + 2 * FD])

                # ---- stage A: joint-mix matmuls (natural x stationary) ----
                pXM = [pmx.tile([128, FD], f32, name="t18", tag=f"pxm{h}")
                       for h in range(2)]
                for i in range(NB):
                    for h in range(2):
                        lhs = xn[:, T * i + 128 * h:T * i + 128 * (h + 1)]
                        nc.tensor.matmul(pXM[h][:, 66 * i:66 * (i + 1)],
                                         lhs, ab[:, 66 * i:66 * (i + 1)],
                                         start=True, stop=True)

                # ---- x4 stream: banded per-node taps (all DVE, bf16 4x) ----
                w3 = [spool.tile([128, FD], bf16, name="t19", tag=f"w3{h}")
                      for h in range(2)]
                w4 = [spool.tile([128, FD], bf16, name="t20", tag=f"w4{h}")
                      for h in range(2)]
                x4s = [spool.tile([128, FD], bf16, name="t21", tag=f"x4s{h}")
                       for h in range(2)]
                x4g = [spool.tile([128, FD], bf16, name="t22", tag=f"x4g{h}")
                       for h in range(2)]
                gX1 = [spool.tile([128, FD], bf16, name="t24", tag=f"gx1{h}")
                       for h in range(2)]
                for h in range(2):
                    nc.vector.tensor_tensor(w3[h][:], xlo[h], c_lo[h][:],
                                            Alu.mult)
                    nc.vector.tensor_tensor(w4[h][:], xhi[h], c_hi[h][:],
                                            Alu.mult)
                    nc.vector.tensor_tensor(x4s[h][:], w3[h][:], w4[h][:],
                                            Alu.add)
                    nc.vector.tensor_tensor(x4g[h][:], x4s[h][:], g3rb[:],
                                            Alu.mult)
                    nc.vector.tensor_tensor(gX1[h][:], sXT[h], g1rb[:],
                                            Alu.mult)

                # ---- streams: s4 = (AL@x)^T + x4g (evac fused) ----
                s4 = [spool.tile([128, FD], bf16, name="t23", tag=f"s4{h}")
                      for h in range(2)]
                for h in range(2):
                    nc.vector.tensor_tensor(s4[h][:], pXM[h][:],
                                            x4g[h][:], Alu.add)

                # ---- stage E: FC via PSUM accumulation ----
                pH = [pph.tile([128, FD], f32, name="t25", tag=f"phh{F}")
                      for F in range(2)]
                for F in range(2):
                    first = True
                    for kh in range(2):
                        nc.tensor.matmul(pH[F][:], c_wq[0][kh][F][:],
                                         s4[kh][:],
                                         start=first, stop=False)
                        first = False
                        nc.tensor.matmul(pH[F][:], c_wq[1][kh][F][:],
                                         gX1[kh][:],
                                         start=False, stop=(kh == 1))

                # ---- stage F: LN stats ----
                ssq = [spool.tile([128, FD], bf16, name="t26", tag=f"ssq{F}")
                       for F in range(2)]
                mr = [stpool.tile([128, NB], f32, name="t27", tag=f"mr{F}")
                      for F in range(2)]
                qr = [stpool.tile([128, NB], f32, name="t28", tag=f"qr{F}")
                      for F in range(2)]
                for F in range(2):
                    nc.vector.tensor_reduce(
                        mr[F][:],
                        pH[F][:].rearrange("p (n v) -> p n v", n=NB),
                        mybir.AxisListType.X, Alu.add)
                    for i in range(NB):
                        nc.scalar.activation(
                            ssq[F][:, 66 * i:66 * (i + 1)],
                            pH[F][:, 66 * i:66 * (i + 1)],
                            Act.Square,
                            accum_out=qr[F][:, i:i + 1])
                mean = [stpool.tile([128, NB], f32, name="t29", tag=f"mean{F}")
                        for F in range(2)]
                rstd = [stpool.tile([128, NB], f32, name="t30", tag=f"rstd{F}")
                        for F in range(2)]
                negm = [stpool.tile([128, NB], f32, name="t31", tag=f"negm{F}")
                        for F in range(2)]
                negmb = [stpool.tile([128, NB], bf16, name="t32",
                                     tag=f"negmb{F}") for F in range(2)]
                tmp = [stpool.tile([128, NB], f32, name="t33", tag=f"tmp{F}")
                       for F in range(2)]
                for F in range(2):
                    nc.gpsimd.tensor_scalar_mul(mean[F][:], mr[F][:], 1.0 / V)
                    nc.gpsimd.tensor_tensor(tmp[F][:], mean[F][:], mean[F][:],
                                            Alu.mult)
                    nc.vector.scalar_tensor_tensor(
                        tmp[F][:], qr[F][:], 1.0 / V, tmp[F][:],
                        Alu.mult, Alu.subtract)
                    nc.scalar.activation(tmp[F][:], tmp[F][:],
                                         Act.Sqrt, bias=c_eps[:])
                    nc.vector.reciprocal(rstd[F][:], tmp[F][:])
                    # negm = (fcb - mean) * rstd
                    nc.vector.scalar_tensor_tensor(
                        negm[F][:], mean[F][:], -1.0,
                        c_fcb[F][:].broadcast_to([128, NB]),
                        Alu.mult, Alu.add)
                    nc.gpsimd.tensor_tensor(negm[F][:], negm[F][:],
                                            rstd[F][:], Alu.mult)
                    nc.scalar.copy(negmb[F][:], negm[F][:])

                # ---- normalize + residual; F=0 on Act, F=1 on DVE ----
                nv = [spool.tile([128, FD], bf16, name="t34", tag=f"nv{F}")
                      for F in range(2)]
                outt = [spool.tile([128, FD], bf16, name="t36", tag=f"outt{F}")
                        for F in range(2)]
                F = 0
                for i in range(NB):
                    nc.scalar.activation(
                        nv[F][:, 66 * i:66 * (i + 1)],
                        pH[F][:, 66 * i:66 * (i + 1)],
                        Act.Identity,
                        bias=negmb[F][:, i:i + 1],
                        scale=rstd[F][:, i:i + 1])
                F = 1
                rbc = rstd[F][:].unsqueeze(2).broadcast_to([128, NB, 66])
                nc.vector.tensor_tensor(
                    nv[F][:].rearrange("p (n v) -> p n v", n=NB),
                    pH[F][:].rearrange("p (n v) -> p n v", n=NB),
                    rbc, Alu.mult)
                nbc = negmb[F][:].unsqueeze(2).broadcast_to([128, NB, 66])
                nc.vector.tensor_tensor(
                    nv[F][:].rearrange("p (n v) -> p n v", n=NB),
                    nv[F][:].rearrange("p (n v) -> p n v", n=NB),
                    nbc, Alu.add)
                for F in range(2):
                    if trivial_affine:
                        nc.vector.tensor_tensor(outt[F][:], nv[F][:],
                                                sXT[F], Alu.add)
                    else:
                        nc.vector.tensor_tensor(nv[F][:], nv[F][:],
                                                c_arep[:], Alu.mult)
                        nc.vector.tensor_tensor(outt[F][:], sXT[F],
                                                c_brep[:], Alu.add)
                        nc.vector.tensor_tensor(outt[F][:], outt[F][:],
                                                nv[F][:], Alu.add)
                    (nc.sync if F == 0 else nc.scalar).dma_start(
                        yst[g, F], outt[F][:])

    nc.compile()
    _NC_CACHE[key] = nc
    return nc


def _gate_np(x, mlp, if_make_dynamic, tau):
    """Replicate the reference gating exactly (jax fp32 on CPU)."""
    import jax
    import jax.numpy as jnp

    xj = jnp.asarray(x)
    prob = xj.mean(axis=1) @ jnp.asarray(mlp)
    if if_make_dynamic:
        u = jax.random.uniform(jax.random.key(42), prob.shape,
                               minval=1e-10, maxval=1.0)
        gumbel = -jnp.log(-jnp.log(u))
        soft = jax.nn.softmax((prob + gumbel) / tau, axis=-1)
        hard = jax.nn.one_hot(jnp.argmax(soft, axis=-1), prob.shape[-1],
                              dtype=soft.dtype)
        gate = hard + soft - soft
    else:
        gate = jnp.zeros_like(prob).at[:, 0].set(1.0)
    return np.asarray(gate, dtype=np.float32)


def _pack_t(xc, BF):
    """[BL, V, T] -> [NG, 2, 128, NB*V] time-on-partition packing."""
    return np.ascontiguousarray(
        xc.reshape(NG, NB, V, 2, 128).transpose(0, 3, 4, 1, 2)
    ).reshape(NG, 2, 128, FD).astype(BF)


def kernel(x, mlp, adj_j, adj_t, adj_jc, adj_tj, fc_w, fc_b, alpha, beta,
           if_make_dynamic, tau):
    from concourse.bass_utils import run_bass_kernel_spmd
    import ml_dtypes

    BF = ml_dtypes.bfloat16

    x = np.asarray(x, dtype=np.float32)
    mlp = np.asarray(mlp, dtype=np.float32)
    adj_j = np.asarray(adj_j, dtype=np.float32)
    adj_t = np.asarray(adj_t, dtype=np.float32)
    adj_jc = np.asarray(adj_jc, dtype=np.float32)
    adj_tj = np.asarray(adj_tj, dtype=np.float32)
    fc_w = np.asarray(fc_w, dtype=np.float32)
    fc_b = np.asarray(fc_b, dtype=np.float32)
    alpha_v = np.asarray(alpha, dtype=np.float32).reshape(V)
    beta_v = np.asarray(beta, dtype=np.float32).reshape(V)
    trivial_affine = bool(np.all(alpha_v == 1.0) and np.all(beta_v == 0.0))

    gate = _gate_np(x, mlp, if_make_dynamic, tau)
    g1, g2, g3 = gate[:, 1], gate[:, 2], gate[:, 3]

    # joint mixing matrices
    A1 = np.kron(adj_j, np.eye(3, dtype=np.float32))          # [66, 66]
    A3 = np.zeros((V, V), dtype=np.float32)                   # block diag
    for j in range(J):
        A3[3 * j:3 * j + 3, 3 * j:3 * j + 3] = adj_jc[j]
    AL = A1[None] + g2[:, None, None] * A3[None]              # [B, 66, 66]
    altT = np.ascontiguousarray(AL.transpose(0, 2, 1))        # [B, 66, 66]

    # banded temporal matrix folded into a second FC matrix
    idx = np.arange(T)
    band = (np.abs(idx[:, None] - idx[None, :]) == 1).astype(np.float32)
    M2 = adj_t * band
    W2 = fc_w @ M2                                            # [T, T]

    # per-node banded tap coefficients (transposed, group-replicated)
    atj_lo = np.zeros((V, T), dtype=np.float32)
    atj_hi = np.zeros((V, T), dtype=np.float32)
    atj_lo[:, 1:] = adj_tj[:, np.arange(1, T), np.arange(0, T - 1)]
    atj_hi[:, :-1] = adj_tj[:, np.arange(0, T - 1), np.arange(1, T)]
    lohi = np.zeros((2, 2, 128, FD), dtype=np.float32)
    for h in range(2):
        lohi[0, h] = np.tile(atj_lo[:, h * 128:(h + 1) * 128].T, (1, NB))
        lohi[1, h] = np.tile(atj_hi[:, h * 128:(h + 1) * 128].T, (1, NB))

    wqs = np.zeros((2, 2, 2, 128, 128), dtype=np.float32)
    for w, M in enumerate((fc_w, W2)):
        for kh in range(2):
            for F in range(2):
                wqs[w, kh, F] = M[128 * F:128 * (F + 1),
                                  128 * kh:128 * (kh + 1)].T
    arep = np.tile(alpha_v[None, :], (128, NB))
    brep = np.tile(beta_v[None, :], (128, NB))
    fcb = np.stack([fc_b[0:128, None], fc_b[128:256, None]]).astype(np.float32)

    lohi_bf = lohi.astype(BF)
    wqs_bf = wqs.astype(BF)
    arep_bf = arep.astype(BF)
    brep_bf = brep.astype(BF)

    in_maps = []
    for cidx in range(N_CORES):
        sl_ = slice(cidx * BL, (cidx + 1) * BL)
        xc = x[sl_]                                           # [BL, V, T]
        xbig = np.empty((NG, 6, 128, FD), dtype=BF)
        xbig[:, 0:2] = _pack_t(xc, BF)
        xbig[:, 2:4] = _pack_t(np.roll(xc, 1, axis=-1), BF)
        xbig[:, 4:6] = _pack_t(np.roll(xc, -1, axis=-1), BF)
        # natural packed + AL^T packed, concatenated along free dim
        xna = np.empty((NG, V, NB * T + FD), dtype=BF)
        xna[:, :, 0:NB * T] = np.ascontiguousarray(
            xc.reshape(NG, NB, V, T).transpose(0, 2, 1, 3)
        ).reshape(NG, V, NB * T)
        xna[:, :, NB * T:] = np.ascontiguousarray(
            altT[sl_].reshape(NG, NB, V, V).transpose(0, 2, 1, 3)
        ).reshape(NG, V, FD)
        g1c, g3c = g1[sl_], g3[sl_]
        gallc = np.zeros((NG, 2, FD), dtype=np.float32)
        gallc[:, 0, :] = np.repeat(g1c.reshape(NG, NB), V, axis=1)
        gallc[:, 1, :] = np.repeat(g3c.reshape(NG, NB), V, axis=1)
        in_maps.append(dict(
            xbig=xbig, xna=xna, gall=gallc.reshape(1, -1).astype(BF),
            lohi=lohi_bf, wqs=wqs_bf, arep=arep_bf, brep=brep_bf, fcb=fcb,
        ))

    nc = _build_nc(trivial_affine)
    res = run_bass_kernel_spmd(nc, in_maps, core_ids=list(range(N_CORES)),
                               **_RUN_KW)
    _LAST_RES.clear()
    _LAST_RES["res"] = res
    out = np.empty((B, V, T), dtype=np.float32)
    for cidx in range(N_CORES):
        yt = np.asarray(res.results[cidx]["yst"])             # [NG,2,128,FD]
        yt = yt.reshape(NG, 2, 128, NB, V).transpose(0, 3, 4, 1, 2)
        out[cidx * BL:(cidx + 1) * BL] = yt.reshape(BL, V, T).astype(np.float32)
    return out


_RUN_KW = {}
_LAST_RES = {}


# revision 17
# speedup vs baseline: 1.1332x; 1.1332x over previous
"""Trainium2 Bass kernel for nn_GCBlock (gnn_message_passing).

Data-parallel over batch (2048 -> 8 cores x 256). Per core, samples are
processed in 32 groups of 8, batched along the free dim in a transposed
(time-on-partition) layout. All matmuls bf16 (1 PE cycle/row), fp32 PSUM.

Key structure per group:
- host ships x natural (PE stationary), x^T, and both banded shifted
  copies of x^T (roll +-1 over t), packed so each group needs only two
  big DMA loads (one 128-partition, one 66-partition) plus two stores.
- joint mixing (A1 + g2*A3) folded on host into per-sample AL (packed
  with natural x), applied as PE matmuls -> (AL@x)^T lands in PSUM.
- g1*x2 (banded adj_t) folded into a second FC matrix W2 = fc_w @ M2band;
  its stream input is g1-gated x^T (one vector op).
- g3*x4 (per-node banded adj_tj) from the pre-shifted x^T copies plus 4
  elementwise ops; summed into the (AL@x)^T stream during PSUM evac.
- FC = PSUM-accumulated matmuls over 2 streams x 2 k-halves.
- LN stats: mean via DVE segmented reduce, E[h^2] via Act square+accum;
  normalize F=0 on Act (per-sample scale/bias), F=1 on DVE broadcast APs.
- residual added in transposed layout; output shipped transposed and
  reassembled (transpose + f32 cast) on host.
"""
import numpy as np

B, V, T, J = 2048, 66, 256, 22
N_CORES = 8
BL = B // N_CORES          # 256 samples per core
NB = 8                     # samples per group
NG = BL // NB              # 32 groups
FD = NB * V                # 528 batched free dim
HC = FD // 2               # 264 per col-half

_NC_CACHE = {}


def _build_nc(trivial_affine):
    key = ("nc", trivial_affine)
    if key in _NC_CACHE:
        return _NC_CACHE[key]
    import concourse.bacc as bacc
    import concourse.mybir as mybir
    import concourse.tile as tile

    f32 = mybir.dt.float32
    bf16 = mybir.dt.bfloat16
    Alu = mybir.AluOpType
    Act = mybir.ActivationFunctionType

    nc = bacc.Bacc("TRN2", target_bir_lowering=False, debug=False,
                   num_devices=N_CORES)

    # packed big inputs: xbig[g] = [xT(h0), xT(h1), xlo(h0), xlo(h1),
    #                               xhi(h0), xhi(h1)] each [128, FD]
    xbig = nc.dram_tensor("xbig", [NG, 6, 128, FD], bf16,
                          kind="ExternalInput").ap()
    # xna[g] = [x natural (NB*T) | AL^T packed (FD)]
    xna = nc.dram_tensor("xna", [NG, V, NB * T + FD], bf16,
                         kind="ExternalInput").ap()
    gall = nc.dram_tensor("gall", [1, NG * 2 * FD], bf16,
                          kind="ExternalInput").ap()
    lohi = nc.dram_tensor("lohi", [2, 2, 128, FD], bf16,
                          kind="ExternalInput").ap()
    wqs = nc.dram_tensor("wqs", [2, 2, 2, 128, 128], bf16,
                         kind="ExternalInput").ap()
    arep = nc.dram_tensor("arep", [128, FD], bf16, kind="ExternalInput").ap()
    brep = nc.dram_tensor("brep", [128, FD], bf16, kind="ExternalInput").ap()
    fcb = nc.dram_tensor("fcb", [2, 128, 1], f32, kind="ExternalInput").ap()
    yst = nc.dram_tensor("yst", [NG, 2, 128, FD], bf16,
                         kind="ExternalOutput").ap()

    with tile.TileContext(nc) as tc:
        import contextlib
        with contextlib.ExitStack() as ctx:
            cpool = ctx.enter_context(tc.tile_pool(name="consts", bufs=1))
            xpool = ctx.enter_context(tc.tile_pool(name="xin", bufs=4))
            spool = ctx.enter_context(tc.tile_pool(name="sbwork", bufs=3))
            stpool = ctx.enter_context(tc.tile_pool(name="stats", bufs=3))
            pmx = ctx.enter_context(tc.tile_pool(name="psA", bufs=1, space="PSUM"))
            pph = ctx.enter_context(tc.tile_pool(name="psH", bufs=1, space="PSUM"))

            # ---- constants ----
            c_lo = [cpool.tile([128, FD], bf16, name=f"clo{h}", tag=f"clo{h}")
                    for h in range(2)]
            c_hi = [cpool.tile([128, FD], bf16, name=f"chi{h}", tag=f"chi{h}")
                    for h in range(2)]
            for h in range(2):
                nc.sync.dma_start(c_lo[h][:], lohi[0, h])
                nc.sync.dma_start(c_hi[h][:], lohi[1, h])
            c_wq = [[[cpool.tile([128, 128], bf16, name=f"cwq{w}{kh}{F}",
                                 tag=f"cwq{w}{kh}{F}")
                      for F in range(2)] for kh in range(2)] for w in range(2)]
            for w in range(2):
                for kh in range(2):
                    for F in range(2):
                        nc.sync.dma_start(c_wq[w][kh][F][:], wqs[w, kh, F])
            c_arep = cpool.tile([128, FD], bf16, name="carep", tag="carep")
            nc.sync.dma_start(c_arep[:], arep[:])
            c_brep = cpool.tile([128, FD], bf16, name="cbrep", tag="cbrep")
            nc.sync.dma_start(c_brep[:], brep[:])
            c_fcb = [cpool.tile([128, 1], f32, name=f"cfcb{F}", tag=f"cfcb{F}")
                     for F in range(2)]
            for F in range(2):
                nc.sync.dma_start(c_fcb[F][:], fcb[F])
            c_eps = cpool.tile([128, 1], f32, name="teps", tag="teps")
            nc.gpsimd.memset(c_eps[:], 1e-5)
            c_gall = cpool.tile([1, NG * 2 * FD], bf16, name="cgall",
                                tag="cgall")
            nc.sync.dma_start(c_gall[:], gall[:])

            for g in range(NG):
                # ---- loads (two big DMAs) ----
                xb = xpool.tile([128, 6 * FD], bf16, name="t10", tag="xb")
                nc.sync.dma_start(
                    xb[:].rearrange("p (s d) -> p s d", s=6),
                    xbig[g].rearrange("s p d -> p s d"))
                xv = xpool.tile([V, NB * T + FD], bf16, name="t11", tag="xv")
                nc.scalar.dma_start(xv[:], xna[g])
                sXT = [xb[:, FD * h:FD * (h + 1)] for h in range(2)]
                xlo = [xb[:, FD * (2 + h):FD * (3 + h)] for h in range(2)]
                xhi = [xb[:, FD * (4 + h):FD * (5 + h)] for h in range(2)]
                xn = xv[:, 0:NB * T]
                ab = xv[:, NB * T:NB * T + FD]

                g1rb = spool.tile([128, FD], bf16, name="t14", tag="g1rb")
                g3rb = spool.tile([128, FD], bf16, name="t15", tag="g3rb")
                goff = g * 2 * FD
                nc.gpsimd.partition_broadcast(
                    g1rb[:], c_gall[:, goff:goff + FD])
                nc.gpsimd.partition_broadcast(
                    g3rb[:], c_gall[:, goff + FD:goff + 2 * FD])

                # ---- stage A: joint-mix matmuls (natural x stationary) ----
                pXM = [[pmx.tile([128, HC], f32, name="t18", tag=f"pxm{h}{c}")
                        for c in range(2)] for h in range(2)]
                for i in range(NB):
                    c, j = i // 4, i % 4
                    for h in range(2):
                        lhs = xn[:, T * i + 128 * h:T * i + 128 * (h + 1)]
                        nc.tensor.matmul(pXM[h][c][:, 66 * j:66 * (j + 1)],
                                         lhs, ab[:, 66 * i:66 * (i + 1)],
                                         start=True, stop=True)

                # ---- x4 stream: banded per-node taps (all DVE, bf16 4x) ----
                w3 = [spool.tile([128, FD], bf16, name="t19", tag=f"w3{h}")
                      for h in range(2)]
                w4 = [spool.tile([128, FD], bf16, name="t20", tag=f"w4{h}")
                      for h in range(2)]
                x4s = [spool.tile([128, FD], bf16, name="t21", tag=f"x4s{h}")
                       for h in range(2)]
                x4g = [spool.tile([128, FD], bf16, name="t22", tag=f"x4g{h}")
                       for h in range(2)]
                gX1 = [spool.tile([128, FD], bf16, name="t24", tag=f"gx1{h}")
                       for h in range(2)]
                for h in range(2):
                    nc.vector.tensor_tensor(w3[h][:], xlo[h], c_lo[h][:],
                                            Alu.mult)
                    nc.vector.tensor_tensor(w4[h][:], xhi[h], c_hi[h][:],
                                            Alu.mult)
                    nc.gpsimd.tensor_tensor(x4s[h][:], w3[h][:], w4[h][:],
                                            Alu.add)
                    nc.gpsimd.tensor_tensor(x4g[h][:], x4s[h][:], g3rb[:],
                                            Alu.mult)
                    nc.vector.tensor_tensor(gX1[h][:], sXT[h], g1rb[:],
                                            Alu.mult)

                # ---- streams: s4 = (AL@x)^T + x4g (evac fused) ----
                s4 = [spool.tile([128, FD], bf16, name="t23", tag=f"s4{h}")
                      for h in range(2)]
                for h in range(2):
                    for c in range(2):
                        sl_ = slice(HC * c, HC * (c + 1))
                        nc.vector.tensor_tensor(s4[h][:, sl_], pXM[h][c][:],
                                                x4g[h][:, sl_], Alu.add)

                # ---- stage E: FC via PSUM accumulation ----
                pH = [[pph.tile([128, HC], f32, name="t25", tag=f"phh{F}{c}")
                       for c in range(2)] for F in range(2)]
                for F in range(2):
                    for c in range(2):
                        sl_ = slice(HC * c, HC * (c + 1))
                        first = True
                        for kh in range(2):
                            nc.tensor.matmul(pH[F][c][:], c_wq[0][kh][F][:],
                                             s4[kh][:, sl_],
                                             start=first, stop=False)
                            first = False
                            nc.tensor.matmul(pH[F][c][:], c_wq[1][kh][F][:],
                                             gX1[kh][:, sl_],
                                             start=False, stop=(kh == 1))

                # ---- stage F: LN stats ----
                ssq = [spool.tile([128, FD], bf16, name="t26", tag=f"ssq{F}")
                       for F in range(2)]
                mr = [stpool.tile([128, NB], f32, name="t27", tag=f"mr{F}")
                      for F in range(2)]
                qr = [stpool.tile([128, NB], f32, name="t28", tag=f"qr{F}")
                      for F in range(2)]
                for F in range(2):
                    for c in range(2):
                        sl_ = slice(HC * c, HC * (c + 1))
                        nc.vector.tensor_reduce(
                            mr[F][:, 4 * c:4 * (c + 1)],
                            pH[F][c][:].rearrange("p (n v) -> p n v", n=4),
                            mybir.AxisListType.X, Alu.add)
                        nc.scalar.square(ssq[F][:, sl_], pH[F][c][:])
                        nc.vector.tensor_reduce(
                            qr[F][:, 4 * c:4 * (c + 1)],
                            ssq[F][:, sl_].rearrange("p (n v) -> p n v", n=4),
                            mybir.AxisListType.X, Alu.add)
                mean = [stpool.tile([128, NB], f32, name="t29", tag=f"mean{F}")
                        for F in range(2)]
                rstd = [stpool.tile([128, NB], f32, name="t30", tag=f"rstd{F}")
                        for F in range(2)]
                negm = [stpool.tile([128, NB], f32, name="t31", tag=f"negm{F}")
                        for F in range(2)]
                negmb = [stpool.tile([128, NB], bf16, name="t32",
                                     tag=f"negmb{F}") for F in range(2)]
                tmp = [stpool.tile([128, NB], f32, name="t33", tag=f"tmp{F}")
                       for F in range(2)]
                for F in range(2):
                    nc.gpsimd.tensor_scalar_mul(mean[F][:], mr[F][:], 1.0 / V)
                    nc.gpsimd.tensor_tensor(tmp[F][:], mean[F][:], mean[F][:],
                                            Alu.mult)
                    nc.vector.scalar_tensor_tensor(
                        tmp[F][:], qr[F][:], 1.0 / V, tmp[F][:],
                        Alu.mult, Alu.subtract)
                    nc.scalar.activation(tmp[F][:], tmp[F][:],
                                         Act.Sqrt, bias=c_eps[:])
                    nc.vector.reciprocal(rstd[F][:], tmp[F][:])
                    # negm = (fcb - mean) * rstd
                    nc.vector.scalar_tensor_tensor(
                        negm[F][:], mean[F][:], -1.0,
                        c_fcb[F][:].broadcast_to([128, NB]),
                        Alu.mult, Alu.add)
                    nc.gpsimd.tensor_tensor(negm[F][:], negm[F][:],
                                            rstd[F][:], Alu.mult)
                    nc.scalar.copy(negmb[F][:], negm[F][:])

                # ---- normalize + residual; F=0 on Act, F=1 on DVE ----
                nv = [spool.tile([128, FD], bf16, name="t34", tag=f"nv{F}")
                      for F in range(2)]
                outt = [spool.tile([128, FD], bf16, name="t36", tag=f"outt{F}")
                        for F in range(2)]
                for F in range(2):
                    for c in range(2):
                        sl_ = slice(HC * c, HC * (c + 1))
                        rbc = rstd[F][:, 4 * c:4 * (c + 1)] \
                            .unsqueeze(2).broadcast_to([128, 4, 66])
                        nc.vector.tensor_tensor(
                            nv[F][:, sl_].rearrange("p (n v) -> p n v", n=4),
                            pH[F][c][:].rearrange("p (n v) -> p n v", n=4),
                            rbc, Alu.mult)
                    nbc = negmb[F][:].unsqueeze(2).broadcast_to([128, NB, 66])
                    nc.gpsimd.tensor_tensor(
                        nv[F][:].rearrange("p (n v) -> p n v", n=NB),
                        nv[F][:].rearrange("p (n v) -> p n v", n=NB),
                        nbc, Alu.add)
                for F in range(2):
                    if trivial_affine:
                        nc.vector.tensor_tensor(outt[F][:], nv[F][:],
                                                sXT[F], Alu.add)
                    else:
                        nc.vector.tensor_tensor(nv[F][:], nv[F][:],
                                                c_arep[:], Alu.mult)
                        nc.vector.tensor_tensor(outt[F][:], sXT[F],
                                                c_brep[:], Alu.add)
                        nc.vector.tensor_tensor(outt[F][:], outt[F][:],
                                                nv[F][:], Alu.add)
                    nc.sync.dma_start(yst[g, F], outt[F][:])

    nc.compile()
    _NC_CACHE[key] = nc
    return nc


def _gate_np(x, mlp, if_make_dynamic, tau):
    """Replicate the reference gating exactly (jax fp32 on CPU)."""
    import jax
    import jax.numpy as jnp

    xj = jnp.asarray(x)
    prob = xj.mean(axis=1) @ jnp.asarray(mlp)
    if if_make_dynamic:
        u = jax.random.uniform(jax.random.key(42), prob.shape,
                               minval=1e-10, maxval=1.0)
        gumbel = -jnp.log(-jnp.log(u))
        soft = jax.nn.softmax((prob + gumbel) / tau, axis=-1)
        hard = jax.nn.one_hot(jnp.argmax(soft, axis=-1), prob.shape[-1],
                              dtype=soft.dtype)
        gate = hard + soft - soft
    else:
        gate = jnp.zeros_like(prob).at[:, 0].set(1.0)
    return np.asarray(gate, dtype=np.float32)


def _pack_t(xc, BF):
    """[BL, V, T] -> [NG, 2, 128, NB*V] time-on-partition packing."""
    return np.ascontiguousarray(
        xc.reshape(NG, NB, V, 2, 128).transpose(0, 3, 4, 1, 2)
    ).reshape(NG, 2, 128, FD).astype(BF)


def kernel(x, mlp, adj_j, adj_t, adj_jc, adj_tj, fc_w, fc_b, alpha, beta,
           if_make_dynamic, tau):
    from concourse.bass_utils import run_bass_kernel_spmd
    import ml_dtypes

    BF = ml_dtypes.bfloat16

    x = np.asarray(x, dtype=np.float32)
    mlp = np.asarray(mlp, dtype=np.float32)
    adj_j = np.asarray(adj_j, dtype=np.float32)
    adj_t = np.asarray(adj_t, dtype=np.float32)
    adj_jc = np.asarray(adj_jc, dtype=np.float32)
    adj_tj = np.asarray(adj_tj, dtype=np.float32)
    fc_w = np.asarray(fc_w, dtype=np.float32)
    fc_b = np.asarray(fc_b, dtype=np.float32)
    alpha_v = np.asarray(alpha, dtype=np.float32).reshape(V)
    beta_v = np.asarray(beta, dtype=np.float32).reshape(V)
    trivial_affine = bool(np.all(alpha_v == 1.0) and np.all(beta_v == 0.0))

    gate = _gate_np(x, mlp, if_make_dynamic, tau)
    g1, g2, g3 = gate[:, 1], gate[:, 2], gate[:, 3]

    # joint mixing matrices
    A1 = np.kron(adj_j, np.eye(3, dtype=np.float32))          # [66, 66]
    A3 = np.zeros((V, V), dtype=np.float32)                   # block diag
    for j in range(J):
        A3[3 * j:3 * j + 3, 3 * j:3 * j + 3] = adj_jc[j]
    AL = A1[None] + g2[:, None, None] * A3[None]              # [B, 66, 66]
    altT = np.ascontiguousarray(AL.transpose(0, 2, 1))        # [B, 66, 66]

    # banded temporal matrix folded into a second FC matrix
    idx = np.arange(T)
    band = (np.abs(idx[:, None] - idx[None, :]) == 1).astype(np.float32)
    M2 = adj_t * band
    W2 = fc_w @ M2                                            # [T, T]

    # per-node banded tap coefficients (transposed, group-replicated)
    atj_lo = np.zeros((V, T), dtype=np.float32)
    atj_hi = np.zeros((V, T), dtype=np.float32)
    atj_lo[:, 1:] = adj_tj[:, np.arange(1, T), np.arange(0, T - 1)]
    atj_hi[:, :-1] = adj_tj[:, np.arange(0, T - 1), np.arange(1, T)]
    lohi = np.zeros((2, 2, 128, FD), dtype=np.float32)
    for h in range(2):
        lohi[0, h] = np.tile(atj_lo[:, h * 128:(h + 1) * 128].T, (1, NB))
        lohi[1, h] = np.tile(atj_hi[:, h * 128:(h + 1) * 128].T, (1, NB))

    wqs = np.zeros((2, 2, 2, 128, 128), dtype=np.float32)
    for w, M in enumerate((fc_w, W2)):
        for kh in range(2):
            for F in range(2):
                wqs[w, kh, F] = M[128 * F:128 * (F + 1),
                                  128 * kh:128 * (kh + 1)].T
    arep = np.tile(alpha_v[None, :], (128, NB))
    brep = np.tile(beta_v[None, :], (128, NB))
    fcb = np.stack([fc_b[0:128, None], fc_b[128:256, None]]).astype(np.float32)

    lohi_bf = lohi.astype(BF)
    wqs_bf = wqs.astype(BF)
    arep_bf = arep.astype(BF)
    brep_bf = brep.astype(BF)

    in_maps = []
    for cidx in range(N_CORES):
        sl_ = slice(cidx * BL, (cidx + 1) * BL)
        xc = x[sl_]                                           # [BL, V, T]
        xbig = np.empty((NG, 6, 128, FD), dtype=BF)
        xbig[:, 0:2] = _pack_t(xc, BF)
        xbig[:, 2:4] = _pack_t(np.roll(xc, 1, axis=-1), BF)
        xbig[:, 4:6] = _pack_t(np.roll(xc, -1, axis=-1), BF)
        # natural packed + AL^T packed, concatenated along free dim
        xna = np.empty((NG, V, NB * T + FD), dtype=BF)
        xna[:, :, 0:NB * T] = np.ascontiguousarray(
            xc.reshape(NG, NB, V, T).transpose(0, 2, 1, 3)
        ).reshape(NG, V, NB * T)
        xna[:, :, NB * T:] = np.ascontiguousarray(
            altT[sl_].reshape(NG, NB, V, V).transpose(0, 2, 1, 3)
        ).reshape(NG, V, FD)
        g1c, g3c = g1[sl_], g3[sl_]
        gallc = np.zeros((NG, 2, FD), dtype=np.float32)
        gallc[:, 0, :] = np.repeat(g1c.reshape(NG, NB), V, axis=1)
        gallc[:, 1, :] = np.repeat(g3c.reshape(NG, NB), V, axis=1)
        in_maps.append(dict(
            xbig=xbig, xna=xna, gall=gallc.reshape(1, -1).astype(BF),
            lohi=lohi_bf, wqs=wqs_bf, arep=arep_bf, brep=brep_bf, fcb=fcb,
        ))

    nc = _build_nc(trivial_affine)
    res = run_bass_kernel_spmd(nc, in_maps, core_ids=list(range(N_CORES)),
                               **_RUN_KW)
    _LAST_RES.clear()
    _LAST_RES["res"] = res
    out = np.empty((B, V, T), dtype=np.float32)
    for cidx in range(N_CORES):
        yt = np.asarray(res.results[cidx]["yst"])             # [NG,2,128,FD]
        yt = yt.reshape(NG, 2, 128, NB, V).transpose(0, 3, 4, 1, 2)
        out[cidx * BL:(cidx + 1) * BL] = yt.reshape(BL, V, T).astype(np.float32)
    return out


_RUN_KW = {}
_LAST_RES = {}


# revision 18
# speedup vs baseline: 1.6391x; 1.4464x over previous
"""Trainium2 Bass kernel for nn_GCBlock (gnn_message_passing).

Data-parallel over batch (2048 -> 8 cores x 256). Per core, samples are
processed in 32 groups of 8, batched along the free dim in a transposed
(time-on-partition) layout. All matmuls bf16 (1 PE cycle/row), fp32 PSUM.

Key structure per group:
- host ships x natural (PE stationary), x^T, and both banded shifted
  copies of x^T (roll +-1 over t), packed so each group needs only two
  big DMA loads (one 128-partition, one 66-partition) plus two stores.
- joint mixing (A1 + g2*A3) folded on host into per-sample AL (packed
  with natural x), applied as PE matmuls -> (AL@x)^T lands in PSUM.
- g1*x2 (banded adj_t) folded into a second FC matrix W2 = fc_w @ M2band;
  its stream input is g1-gated x^T (one vector op).
- g3*x4 (per-node banded adj_tj) from the pre-shifted x^T copies plus 4
  elementwise ops; summed into the (AL@x)^T stream during PSUM evac.
- FC = PSUM-accumulated matmuls over 2 streams x 2 k-halves.
- LN stats: mean via DVE segmented reduce, E[h^2] via Act square+accum;
  normalize F=0 on Act (per-sample scale/bias), F=1 on DVE broadcast APs.
- residual added in transposed layout; output shipped transposed and
  reassembled (transpose + f32 cast) on host.
"""
import numpy as np

B, V, T, J = 2048, 66, 256, 22
N_CORES = 8
BL = B // N_CORES          # 256 samples per core
NB = 8                     # samples per group
NG = BL // NB              # 32 groups
FD = NB * V                # 528 batched free dim
HC = FD // 2               # 264 per col-half

_NC_CACHE = {}


def _build_nc(trivial_affine):
    key = ("nc", trivial_affine)
    if key in _NC_CACHE:
        return _NC_CACHE[key]
    import concourse.bacc as bacc
    import concourse.mybir as mybir
    import concourse.tile as tile

    f32 = mybir.dt.float32
    bf16 = mybir.dt.bfloat16
    Alu = mybir.AluOpType
    Act = mybir.ActivationFunctionType

    nc = bacc.Bacc("TRN2", target_bir_lowering=False, debug=False,
                   num_devices=N_CORES)

    # packed big inputs: xbig[g] = [xT(h0), xT(h1), xlo(h0), xlo(h1),
    #                               xhi(h0), xhi(h1)] each [128, FD]
    xbig = nc.dram_tensor("xbig", [NG, 6, 128, FD], bf16,
                          kind="ExternalInput").ap()
    # xna[g] = [x natural (NB*T) | AL^T packed (FD)]
    xna = nc.dram_tensor("xna", [NG, V, NB * T + FD], bf16,
                         kind="ExternalInput").ap()
    gall = nc.dram_tensor("gall", [1, NG * 2 * FD], bf16,
                          kind="ExternalInput").ap()
    lohi = nc.dram_tensor("lohi", [2, 2, 128, FD], bf16,
                          kind="ExternalInput").ap()
    wqs = nc.dram_tensor("wqs", [2, 2, 2, 128, 128], bf16,
                         kind="ExternalInput").ap()
    arep = nc.dram_tensor("arep", [128, FD], bf16, kind="ExternalInput").ap()
    brep = nc.dram_tensor("brep", [128, FD], bf16, kind="ExternalInput").ap()
    fcb = nc.dram_tensor("fcb", [2, 128, 1], f32, kind="ExternalInput").ap()
    yst = nc.dram_tensor("yst", [NG, 2, 128, FD], bf16,
                         kind="ExternalOutput").ap()

    with tile.TileContext(nc) as tc:
        import contextlib
        with contextlib.ExitStack() as ctx:
            cpool = ctx.enter_context(tc.tile_pool(name="consts", bufs=1))
            xpool = ctx.enter_context(tc.tile_pool(name="xin", bufs=4))
            spool = ctx.enter_context(tc.tile_pool(name="sbwork", bufs=3))
            stpool = ctx.enter_context(tc.tile_pool(name="stats", bufs=3))
            pmx = ctx.enter_context(tc.tile_pool(name="psA", bufs=2, space="PSUM"))
            pph = ctx.enter_context(tc.tile_pool(name="psH", bufs=2, space="PSUM"))

            # ---- constants ----
            c_lo = [cpool.tile([128, FD], bf16, name=f"clo{h}", tag=f"clo{h}")
                    for h in range(2)]
            c_hi = [cpool.tile([128, FD], bf16, name=f"chi{h}", tag=f"chi{h}")
                    for h in range(2)]
            for h in range(2):
                nc.sync.dma_start(c_lo[h][:], lohi[0, h])
                nc.sync.dma_start(c_hi[h][:], lohi[1, h])
            c_wq = [[[cpool.tile([128, 128], bf16, name=f"cwq{w}{kh}{F}",
                                 tag=f"cwq{w}{kh}{F}")
                      for F in range(2)] for kh in range(2)] for w in range(2)]
            for w in range(2):
                for kh in range(2):
                    for F in range(2):
                        nc.sync.dma_start(c_wq[w][kh][F][:], wqs[w, kh, F])
            c_arep = cpool.tile([128, FD], bf16, name="carep", tag="carep")
            nc.sync.dma_start(c_arep[:], arep[:])
            c_brep = cpool.tile([128, FD], bf16, name="cbrep", tag="cbrep")
            nc.sync.dma_start(c_brep[:], brep[:])
            c_fcb = [cpool.tile([128, 1], f32, name=f"cfcb{F}", tag=f"cfcb{F}")
                     for F in range(2)]
            for F in range(2):
                nc.sync.dma_start(c_fcb[F][:], fcb[F])
            c_eps = cpool.tile([128, 1], f32, name="teps", tag="teps")
            nc.gpsimd.memset(c_eps[:], 1e-5)
            c_gall = cpool.tile([1, NG * 2 * FD], bf16, name="cgall",
                                tag="cgall")
            nc.sync.dma_start(c_gall[:], gall[:])

            for g in range(NG):
                # ---- loads (two big DMAs) ----
                xb = xpool.tile([128, 6 * FD], bf16, name="t10", tag="xb")
                nc.sync.dma_start(
                    xb[:].rearrange("p (s d) -> p s d", s=6),
                    xbig[g].rearrange("s p d -> p s d"))
                xv = xpool.tile([V, NB * T + FD], bf16, name="t11", tag="xv")
                nc.scalar.dma_start(xv[:], xna[g])
                sXT = [xb[:, FD * h:FD * (h + 1)] for h in range(2)]
                xlo = [xb[:, FD * (2 + h):FD * (3 + h)] for h in range(2)]
                xhi = [xb[:, FD * (4 + h):FD * (5 + h)] for h in range(2)]
                xn = xv[:, 0:NB * T]
                ab = xv[:, NB * T:NB * T + FD]

                g1rb = spool.tile([128, FD], bf16, name="t14", tag="g1rb")
                g3rb = spool.tile([128, FD], bf16, name="t15", tag="g3rb")
                goff = g * 2 * FD
                nc.gpsimd.partition_broadcast(
                    g1rb[:], c_gall[:, goff:goff + FD])
                nc.gpsimd.partition_broadcast(
                    g3rb[:], c_gall[:, goff + FD:goff + 2 * FD])

                # ---- stage A: joint-mix matmuls (natural x stationary) ----
                pXM = [[pmx.tile([128, HC], f32, name="t18", tag=f"pxm{h}")
                        for c in range(2)] for h in range(2)]
                for i in range(NB):
                    c, j = i // 4, i % 4
                    for h in range(2):
                        lhs = xn[:, T * i + 128 * h:T * i + 128 * (h + 1)]
                        nc.tensor.matmul(pXM[h][c][:, 66 * j:66 * (j + 1)],
                                         lhs, ab[:, 66 * i:66 * (i + 1)],
                                         start=True, stop=True)

                # ---- x4 stream: banded per-node taps (all DVE, bf16 4x) ----
                w3 = [spool.tile([128, FD], bf16, name="t19", tag=f"w3{h}")
                      for h in range(2)]
                w4 = [spool.tile([128, FD], bf16, name="t20", tag=f"w4{h}")
                      for h in range(2)]
                x4s = [spool.tile([128, FD], bf16, name="t21", tag=f"x4s{h}")
                       for h in range(2)]
                x4g = [spool.tile([128, FD], bf16, name="t22", tag=f"x4g{h}")
                       for h in range(2)]
                gX1 = [spool.tile([128, FD], bf16, name="t24", tag=f"gx1{h}")
                       for h in range(2)]
                for h in range(2):
                    nc.vector.tensor_tensor(w3[h][:], xlo[h], c_lo[h][:],
                                            Alu.mult)
                    nc.vector.tensor_tensor(w4[h][:], xhi[h], c_hi[h][:],
                                            Alu.mult)
                    nc.vector.tensor_tensor(x4s[h][:], w3[h][:], w4[h][:],
                                            Alu.add)
                    nc.vector.tensor_tensor(x4g[h][:], x4s[h][:], g3rb[:],
                                            Alu.mult)
                    nc.vector.tensor_tensor(gX1[h][:], sXT[h], g1rb[:],
                                            Alu.mult)

                # ---- streams: s4 = (AL@x)^T + x4g (evac fused) ----
                s4 = [spool.tile([128, FD], bf16, name="t23", tag=f"s4{h}")
                      for h in range(2)]
                for h in range(2):
                    for c in range(2):
                        sl_ = slice(HC * c, HC * (c + 1))
                        nc.vector.tensor_tensor(s4[h][:, sl_], pXM[h][c][:],
                                                x4g[h][:, sl_], Alu.add)

                # ---- stage E: FC via PSUM accumulation; evac to SBUF ----
                hb = [spool.tile([128, FD], bf16, name="t25h", tag=f"hb{F}")
                      for F in range(2)]
                for F in range(2):
                    for c in range(2):
                        sl_ = slice(HC * c, HC * (c + 1))
                        pHt = pph.tile([128, HC], f32, name="t25",
                                       tag=f"phh{F}")
                        first = True
                        for kh in range(2):
                            nc.tensor.matmul(pHt[:], c_wq[0][kh][F][:],
                                             s4[kh][:, sl_],
                                             start=first, stop=False)
                            first = False
                            nc.tensor.matmul(pHt[:], c_wq[1][kh][F][:],
                                             gX1[kh][:, sl_],
                                             start=False, stop=(kh == 1))
                        nc.scalar.copy(hb[F][:, sl_], pHt[:])

                # ---- stage F: LN stats ----
                ssq = [spool.tile([128, FD], bf16, name="t26", tag=f"ssq{F}")
                       for F in range(2)]
                mr = [stpool.tile([128, NB], f32, name="t27", tag=f"mr{F}")
                      for F in range(2)]
                qr = [stpool.tile([128, NB], f32, name="t28", tag=f"qr{F}")
                      for F in range(2)]
                for F in range(2):
                    nc.scalar.square(ssq[F][:], hb[F][:])
                    for c in range(2):
                        sl_ = slice(HC * c, HC * (c + 1))
                        nc.vector.tensor_reduce(
                            mr[F][:, 4 * c:4 * (c + 1)],
                            hb[F][:, sl_].rearrange("p (n v) -> p n v", n=4),
                            mybir.AxisListType.X, Alu.add)
                        nc.vector.tensor_reduce(
                            qr[F][:, 4 * c:4 * (c + 1)],
                            ssq[F][:, sl_].rearrange("p (n v) -> p n v", n=4),
                            mybir.AxisListType.X, Alu.add)
                mean = [stpool.tile([128, NB], f32, name="t29", tag=f"mean{F}")
                        for F in range(2)]
                rstd = [stpool.tile([128, NB], f32, name="t30", tag=f"rstd{F}")
                        for F in range(2)]
                negm = [stpool.tile([128, NB], f32, name="t31", tag=f"negm{F}")
                        for F in range(2)]
                negmb = [stpool.tile([128, NB], bf16, name="t32",
                                     tag=f"negmb{F}") for F in range(2)]
                tmp = [stpool.tile([128, NB], f32, name="t33", tag=f"tmp{F}")
                       for F in range(2)]
                for F in range(2):
                    nc.gpsimd.tensor_scalar_mul(mean[F][:], mr[F][:], 1.0 / V)
                    nc.gpsimd.tensor_tensor(tmp[F][:], mean[F][:], mean[F][:],
                                            Alu.mult)
                    nc.vector.scalar_tensor_tensor(
                        tmp[F][:], qr[F][:], 1.0 / V, tmp[F][:],
                        Alu.mult, Alu.subtract)
                    nc.scalar.activation(tmp[F][:], tmp[F][:],
                                         Act.Sqrt, bias=c_eps[:])
                    nc.vector.reciprocal(rstd[F][:], tmp[F][:])
                    # negm = (fcb - mean) * rstd
                    nc.vector.scalar_tensor_tensor(
                        negm[F][:], mean[F][:], -1.0,
                        c_fcb[F][:].broadcast_to([128, NB]),
                        Alu.mult, Alu.add)
                    nc.gpsimd.tensor_tensor(negm[F][:], negm[F][:],
                                            rstd[F][:], Alu.mult)
                    nc.scalar.copy(negmb[F][:], negm[F][:])

                # ---- normalize + residual; F=0 on Act, F=1 on DVE ----
                nv = [spool.tile([128, FD], bf16, name="t34", tag=f"nv{F}")
                      for F in range(2)]
                outt = [spool.tile([128, FD], bf16, name="t36", tag=f"outt{F}")
                        for F in range(2)]
                for F in range(2):
                    rbc = rstd[F][:].unsqueeze(2).broadcast_to([128, NB, 66])
                    nc.vector.tensor_tensor(
                        nv[F][:].rearrange("p (n v) -> p n v", n=NB),
                        hb[F][:].rearrange("p (n v) -> p n v", n=NB),
                        rbc, Alu.mult)
                    nbc = negmb[F][:].unsqueeze(2).broadcast_to([128, NB, 66])
                    nc.vector.tensor_tensor(
                        nv[F][:].rearrange("p (n v) -> p n v", n=NB),
                        nv[F][:].rearrange("p (n v) -> p n v", n=NB),
                        nbc, Alu.add)
                for F in range(2):
                    if trivial_affine:
                        nc.vector.tensor_tensor(outt[F][:], nv[F][:],
                                                sXT[F], Alu.add)
                    else:
                        nc.vector.tensor_tensor(nv[F][:], nv[F][:],
                                                c_arep[:], Alu.mult)
                        nc.vector.tensor_tensor(outt[F][:], sXT[F],
                                                c_brep[:], Alu.add)
                        nc.vector.tensor_tensor(outt[F][:], outt[F][:],
                                                nv[F][:], Alu.add)
                    nc.sync.dma_start(yst[g, F], outt[F][:])

    nc.compile()
    _NC_CACHE[key] = nc
    return nc


def _gate_np(x, mlp, if_make_dynamic, tau):
    """Replicate the reference gating exactly (jax fp32 on CPU)."""
    import jax
    import jax.numpy as jnp

    xj = jnp.asarray(x)
    prob = xj.mean(axis=1) @ jnp.asarray(mlp)
    if if_make_dynamic:
        u = jax.random.uniform(jax.random.key(42), prob.shape,
                               minval=1e-10, maxval=1.0)
        gumbel = -jnp.log(-jnp.log(u))
        soft = jax.nn.softmax((prob + gumbel) / tau, axis=-1)
        hard = jax.nn.one_hot(jnp.argmax(soft, axis=-1), prob.shape[-1],
                              dtype=soft.dtype)
        gate = hard + soft - soft
    else:
        gate = jnp.zeros_like(prob).at[:, 0].set(1.0)
    return np.asarray(gate, dtype=np.float32)


def _pack_t(xc, BF):
    """[BL, V, T] -> [NG, 2, 128, NB*V] time-on-partition packing."""
    return np.ascontiguousarray(
        xc.reshape(NG, NB, V, 2, 128).transpose(0, 3, 4, 1, 2)
    ).reshape(NG, 2, 128, FD).astype(BF)


def kernel(x, mlp, adj_j, adj_t, adj_jc, adj_tj, fc_w, fc_b, alpha, beta,
           if_make_dynamic, tau):
    from concourse.bass_utils import run_bass_kernel_spmd
    import ml_dtypes

    BF = ml_dtypes.bfloat16

    x = np.asarray(x, dtype=np.float32)
    mlp = np.asarray(mlp, dtype=np.float32)
    adj_j = np.asarray(adj_j, dtype=np.float32)
    adj_t = np.asarray(adj_t, dtype=np.float32)
    adj_jc = np.asarray(adj_jc, dtype=np.float32)
    adj_tj = np.asarray(adj_tj, dtype=np.float32)
    fc_w = np.asarray(fc_w, dtype=np.float32)
    fc_b = np.asarray(fc_b, dtype=np.float32)
    alpha_v = np.asarray(alpha, dtype=np.float32).reshape(V)
    beta_v = np.asarray(beta, dtype=np.float32).reshape(V)
    trivial_affine = bool(np.all(alpha_v == 1.0) and np.all(beta_v == 0.0))

    gate = _gate_np(x, mlp, if_make_dynamic, tau)
    g1, g2, g3 = gate[:, 1], gate[:, 2], gate[:, 3]

    # joint mixing matrices
    A1 = np.kron(adj_j, np.eye(3, dtype=np.float32))          # [66, 66]
    A3 = np.zeros((V, V), dtype=np.float32)                   # block diag
    for j in range(J):
        A3[3 * j:3 * j + 3, 3 * j:3 * j + 3] = adj_jc[j]
    AL = A1[None] + g2[:, None, None] * A3[None]              # [B, 66, 66]
    altT = np.ascontiguousarray(AL.transpose(0, 2, 1))        # [B, 66, 66]

    # banded temporal matrix folded into a second FC matrix
    idx = np.arange(T)
    band = (np.abs(idx[:, None] - idx[None, :]) == 1).astype(np.float32)
    M2 = adj_t * band
    W2 = fc_w @ M2                                            # [T, T]

    # per-node banded tap coefficients (transposed, group-replicated)
    atj_lo = np.zeros((V, T), dtype=np.float32)
    atj_hi = np.zeros((V, T), dtype=np.float32)
    atj_lo[:, 1:] = adj_tj[:, np.arange(1, T), np.arange(0, T - 1)]
    atj_hi[:, :-1] = adj_tj[:, np.arange(0, T - 1), np.arange(1, T)]
    lohi = np.zeros((2, 2, 128, FD), dtype=np.float32)
    for h in range(2):
        lohi[0, h] = np.tile(atj_lo[:, h * 128:(h + 1) * 128].T, (1, NB))
        lohi[1, h] = np.tile(atj_hi[:, h * 128:(h + 1) * 128].T, (1, NB))

    wqs = np.zeros((2, 2, 2, 128, 128), dtype=np.float32)
    for w, M in enumerate((fc_w, W2)):
        for kh in range(2):
            for F in range(2):
                wqs[w, kh, F] = M[128 * F:128 * (F + 1),
                                  128 * kh:128 * (kh + 1)].T
    arep = np.tile(alpha_v[None, :], (128, NB))
    brep = np.tile(beta_v[None, :], (128, NB))
    fcb = np.stack([fc_b[0:128, None], fc_b[128:256, None]]).astype(np.float32)

    lohi_bf = lohi.astype(BF)
    wqs_bf = wqs.astype(BF)
    arep_bf = arep.astype(BF)
    brep_bf = brep.astype(BF)

    in_maps = []
    for cidx in range(N_CORES):
        sl_ = slice(cidx * BL, (cidx + 1) * BL)
        xc = x[sl_]                                           # [BL, V, T]
        xbig = np.empty((NG, 6, 128, FD), dtype=BF)
        xbig[:, 0:2] = _pack_t(xc, BF)
        xbig[:, 2:4] = _pack_t(np.roll(xc, 1, axis=-1), BF)
        xbig[:, 4:6] = _pack_t(np.roll(xc, -1, axis=-1), BF)
        # natural packed + AL^T packed, concatenated along free dim
        xna = np.empty((NG, V, NB * T + FD), dtype=BF)
        xna[:, :, 0:NB * T] = np.ascontiguousarray(
            xc.reshape(NG, NB, V, T).transpose(0, 2, 1, 3)
        ).reshape(NG, V, NB * T)
        xna[:, :, NB * T:] = np.ascontiguousarray(
            altT[sl_].reshape(NG, NB, V, V).transpose(0, 2, 1, 3)
        ).reshape(NG, V, FD)
        g1c, g3c = g1[sl_], g3[sl_]
        gallc = np.zeros((NG, 2, FD), dtype=np.float32)
        gallc[:, 0, :] = np.repeat(g1c.reshape(NG, NB), V, axis=1)
        gallc[:, 1, :] = np.repeat(g3c.reshape(NG, NB), V, axis=1)
        in_maps.append(dict(
            xbig=xbig, xna=xna, gall=gallc.reshape(1, -1).astype(BF),
            lohi=lohi_bf, wqs=wqs_bf, arep=arep_bf, brep=brep_bf, fcb=fcb,
        ))

    nc = _build_nc(trivial_affine)
    res = run_bass_kernel_spmd(nc, in_maps, core_ids=list(range(N_CORES)),
                               **_RUN_KW)
    _LAST_RES.clear()
    _LAST_RES["res"] = res
    out = np.empty((B, V, T), dtype=np.float32)
    for cidx in range(N_CORES):
        yt = np.asarray(res.results[cidx]["yst"])             # [NG,2,128,FD]
        yt = yt.reshape(NG, 2, 128, NB, V).transpose(0, 3, 4, 1, 2)
        out[cidx * BL:(cidx + 1) * BL] = yt.reshape(BL, V, T).astype(np.float32)
    return out


_RUN_KW = {}
_LAST_RES = {}
